# revision 2
# baseline (speedup 1.0000x reference)
"""CSRN Trainium2 kernel v2: chunked-parallel scan.

Sharding: data-parallel over batch across 8 cores (4 batches/core).
Each core fuses both directional scans; the 512 scan steps are split
into 8 chunks run in parallel (the recurrence is contractive: W=16
warmup steps from h=0 reproduce the exact state to ~1e-6).

Organization per core: 2 independent instruction streams; each stream
carries 32 chains (4 chunks x [2 scans x 4 batches]) x 3 channels =
96 data rows, positions 0..511 on the free dim. slots/stream =
S/8 + W.  Gate matmuls: M-tiles r/z/inn/hnh (96 rows each); inn has
no h-part and hnh no x-part, so 6 matmuls/slot + 3 conv taps.

Layout rules honored: engine SBUF access patterns start at partition
0/32/64/96 only (PSUM operands exempt); matmul lhsT/rhs share a
32-aligned base partition; one matmul output <= one PSUM bank (N<=512).
"""

import sys

sys.path.insert(0, "/opt/trn_rl_repo")

import numpy as np
import ml_dtypes

BF16 = ml_dtypes.bfloat16

C = 3  # channels
B = 4  # batches per core
NCORES = 8
NCH = 8  # global chunks
G = 2  # streams
JPG = NCH // G  # chunks per stream
BS = 2 * B  # batch-scan chains (scan 0 = rows, 1 = cols)
R = JPG * BS * C  # 96 data rows per stream
W = 8  # warmup steps
XRING = 12  # x ring slots
XCH = 4  # x DMA chunk (slots)
HS = 2  # h ring slots


def _pack_weights(inp):
    """lhsT matrices -> single (128, 1160) bf16 block 'wpk'.

    Row order within a stream: row = 24*j + 3*bs + c, j = chunk-in-stream,
    bs = scan*4 + b (scan 0 = row scan), c = channel.
    Column map: wgA (x+ones rows, 4 gate M-tiles) 0:384; wgB (h rows)
    384:768; conv taps 768:1056; wa 1056:1104; wl 1104:1152;
    conv bias col 1152; combine bias col 1153.
    """
    w_ih = [inp["w_ih_rows"], inp["w_ih_cols"]]
    w_hh = [inp["w_hh_rows"], inp["w_hh_cols"]]
    b_ih = [inp["b_ih_rows"], inp["b_ih_cols"]]
    b_hh = [inp["b_hh_rows"], inp["b_hh_cols"]]
    cv_w = [inp["conv_rows_w"], inp["conv_cols_w"]]
    cv_b = [inp["conv_rows_b"], inp["conv_cols_b"]]
    cb_w = inp["combine_w"]  # (C, 2C)
    cb_b = inp["combine_b"]  # (C,)

    wgA = np.zeros((97, 4 * R), np.float32)  # K = x rows 0..95, ones 96
    wgB = np.zeros((97, 4 * R), np.float32)  # K = h rows 0..95, ones 96
    # gate order in M-tiles: 0=r, 1=z, 2=inn, 3=hnh
    for j in range(JPG):
        for bs in range(BS):
            sc = bs // B
            base = 24 * j + 3 * bs
            for ci in range(C):
                for co in range(C):
                    k, m = base + ci, base + co
                    wgA[k, 0 * R + m] = w_ih[sc][0 + co, ci]
                    wgB[k, 0 * R + m] = w_hh[sc][0 + co, ci]
                    wgA[k, 1 * R + m] = w_ih[sc][3 + co, ci]
                    wgB[k, 1 * R + m] = w_hh[sc][3 + co, ci]
                    wgA[k, 2 * R + m] = w_ih[sc][6 + co, ci]
                    wgB[k, 3 * R + m] = w_hh[sc][6 + co, ci]
            for co in range(C):
                m = base + co
                wgA[96, 0 * R + m] = b_ih[sc][co] + b_hh[sc][co]
                wgA[96, 1 * R + m] = b_ih[sc][3 + co] + b_hh[sc][3 + co]
                wgA[96, 2 * R + m] = b_ih[sc][6 + co]
                wgB[96, 3 * R + m] = b_hh[sc][6 + co]

    wcv = np.zeros((R, 3, R), np.float32)
    for j in range(JPG):
        for bs in range(BS):
            sc = bs // B
            base = 24 * j + 3 * bs
            for t in range(3):
                for ci in range(C):
                    for co in range(C):
                        wcv[base + ci, t, base + co] = cv_w[sc][co, ci, t]

    # combine lhsT: K = 96 ctx rows, M = 48 (j, b, o)
    wa = np.zeros((R, 48), np.float32)
    wl = np.zeros((R, 48), np.float32)
    for j in range(JPG):
        for b in range(B):
            for o in range(C):
                m = 12 * j + 3 * b + o
                for ci in range(C):
                    wa[24 * j + 3 * b + ci, m] = cb_w[o, ci]
                    wl[24 * j + 3 * (4 + b) + ci, m] = cb_w[o, C + ci]

    wpk = np.zeros((128, 1256), np.float32)
    wpk[0:48, 1155:1203] = np.eye(48, dtype=np.float32)
    wpk[0:97, 0:384] = wgA
    wpk[0:97, 384:768] = wgB
    wpk[0:R, 768:1056] = wcv.reshape(R, 288)
    wpk[0:R, 1056:1104] = wa
    wpk[0:R, 1104:1152] = wl
    for j in range(JPG):
        for bs in range(BS):
            for co in range(C):
                wpk[24 * j + 3 * bs + co, 1152] = cv_b[bs // B][co]
    for j in range(JPG):
        for b in range(B):
            for o in range(C):
                wpk[12 * j + 3 * b + o, 1153] = cb_b[o]
                for p in range(2):
                    wpk[64 * p + 12 * j + 3 * b + o, 1154] = cb_b[o]
    return {"wpk": wpk.astype(BF16)}


def _pack_x2(x_core, S):
    """x_core: (B, C, S, S) f32 -> (G, 97, slots, S) bf16 matmul rows.

    x2[g, 24j+3bs+c, t, :] = x[b, c, step, :] (row scan, bs<4) or
    x[b, c, :, step] (col scan), step = (S/8)*(JPG*g+j) + t - W; zeros
    for step < 0 (chunk 0 warmup; its h is reset at slot W on device).
    Row 96 = ones.
    """
    CL = S // NCH
    slots = CL + W
    x2 = np.zeros((G, 96, slots, S), np.float32)
    steps = np.arange(slots) - W
    for g in range(G):
        for j in range(JPG):
            cg = JPG * g + j
            st = CL * cg + steps
            valid = st >= 0
            stc = np.clip(st, 0, S - 1)
            for bs in range(BS):
                b = bs % B
                rows = slice(24 * j + 3 * bs, 24 * j + 3 * bs + 3)
                if bs < 4:
                    blk = x_core[b, :, stc, :]  # (slots, C, S)
                else:
                    blk = x_core[b, :, :, stc]  # (slots, C, S)
                blk = np.transpose(blk, (1, 0, 2)) * valid[None, :, None]
                x2[g, rows, :, :] = blk
    return x2.astype(BF16)


def build_nc(S, lt_pmajor=False):
    import concourse.bass as bass
    import concourse.bacc as bacc
    import concourse.mybir as mybir
    from concourse.tile import TileContext
    from contextlib import ExitStack

    fp32 = mybir.dt.float32
    bf16 = mybir.dt.bfloat16
    AF = mybir.ActivationFunctionType
    OP = mybir.AluOpType

    CL = S // NCH  # chunk length (real slots per stream)
    SLOTS = CL + W

    nc = bacc.Bacc()
    x2_d = nc.declare_dram_parameter("x2", [G, 96, SLOTS, S], bf16, isOutput=False)
    wpk_d = nc.declare_dram_parameter("wpk", [128, 1256], bf16, isOutput=False)
    out_d = nc.declare_dram_parameter("out", [G, S // (4 * NCH), 2, 48, 2, S], bf16, isOutput=True)
    lp_d = nc.dram_tensor("lpart", [2, B, C, S, S // 2], bf16)  # [hpar, b, o, w, h2]
    lpt_d = nc.dram_tensor("lpartT", [2, B, C, S // 2, S], bf16)  # [hpar, b, o, h2, w]

    lp_v = lp_d.rearrange("p b o (c wl) h2 -> p c b o wl h2", c=NCH)
    lpt_p4 = lpt_d.rearrange("p b o (c q4 qq) w -> p c b o q4 qq w", c=NCH, qq=2)

    with TileContext(nc) as tc, ExitStack() as es:
        cst = es.enter_context(tc.tile_pool(name="cst", bufs=1))
        big = es.enter_context(tc.tile_pool(name="big", bufs=1))
        wrk = es.enter_context(tc.tile_pool(name="wrk", bufs=2))
        pss = [
            es.enter_context(tc.tile_pool(name=f"ps{g}", bufs=1, space="PSUM"))
            for g in range(G)
        ]

        wpk = cst.tile([128, 1256], bf16)
        nc.sync.dma_start(out=wpk[:], in_=wpk_d[:])
        wg = [wpk[0:97, 0:384], wpk[0:97, 384:768]]  # A (x+ones), B (h+ones)
        wcv = wpk[0:R, 768:1056].rearrange("p (t m) -> p t m", t=3)
        wa = wpk[0:R, 1056:1104]
        wl = wpk[0:R, 1104:1152]
        cvb = wpk[0:R, 1152:1153]
        cbb = wpk[0:48, 1153:1154]
        idt = wpk[0:48, 1155:1203]

        # persistent state per stream
        xts = [big.tile([128, XRING, S], bf16, name=f"xts{g}") for g in range(G)]
        hh = [big.tile([97, HS, S], bf16, name=f"hh{g}") for g in range(G)]
        ctx = [big.tile([R, CL, S + 2], bf16, name=f"ctx{g}") for g in range(G)]
        for g in range(G):
            nc.vector.memset(hh[g][:], 0.0)
            nc.vector.memset(hh[g][96:97, :, :], 1.0)
            nc.vector.memset(xts[g][96:97, :, :], 1.0)
            nc.vector.memset(ctx[g][:, :, 0:1], 0.0)
            nc.vector.memset(ctx[g][:, :, S + 1 : S + 2], 0.0)
            # initial x prefetch: slots [0, 2*XCH)
            for t0 in (0, XCH):
                nc.sync.dma_start(
                    out=xts[g][0:96, t0 : t0 + XCH, :],
                    in_=x2_d[g, :, t0 : t0 + XCH, :],
                )

        # ---------------- scan loop ----------------
        for t in range(SLOTS):
            for g in range(G):
                ps = pss[g]
                sl = t % XRING
                hsl = t % HS
                cslot = t - W if t >= W else min(t, CL - 1)
                if t % XCH == 0:
                    t0 = t + 2 * XCH
                    if t0 < SLOTS:
                        n_s = min(XCH, SLOTS - t0)
                        nc.sync.dma_start(
                            out=xts[g][0:96, t0 % XRING : t0 % XRING + n_s, :],
                            in_=x2_d[g, :, t0 : t0 + n_s, :],
                        )
                xrow = xts[g][0:97, sl, :]
                hrow = hh[g][0:97, hsl, :]
                hrow0 = hh[g][0:R, hsl, :]

                pr = ps.tile([R, S], fp32, tag="pr", name=f"pr{g}")
                pz = ps.tile([R, S], fp32, tag="pz", name=f"pz{g}")
                pinn = ps.tile([R, S], fp32, tag="pinn", name=f"pinn{g}")
                pq = ps.tile([R, S], fp32, tag="pq", name=f"pq{g}")  # hnh
                # gates: r, z need x+h; inn x-only; hnh h-only
                nc.tensor.matmul(pr[:], wg[0][:, 0 * R : 0 * R + R], xrow, start=True, stop=False)
                nc.tensor.matmul(pr[:], wg[1][:, 0 * R : 0 * R + R], hrow, start=False, stop=True)
                nc.tensor.matmul(pz[:], wg[0][:, 1 * R : 1 * R + R], xrow, start=True, stop=False)
                nc.tensor.matmul(pz[:], wg[1][:, 1 * R : 1 * R + R], hrow, start=False, stop=True)
                nc.tensor.matmul(pinn[:], wg[0][:, 2 * R : 2 * R + R], xrow, start=True, stop=True)
                nc.tensor.matmul(pq[:], wg[1][:, 3 * R : 3 * R + R], hrow, start=True, stop=True)

                rsb = wrk.tile([R, S], bf16, tag=f"rsb{g}", bufs=1)
                zsb = wrk.tile([R, S], bf16, tag=f"zsb{g}", bufs=1)
                nc.scalar.activation(rsb[:], pr[:], AF.Sigmoid)
                nc.scalar.activation(zsb[:], pz[:], AF.Sigmoid)
                # n-path: pinn += r * hnh ; n = tanh(pinn)
                t1 = wrk.tile([R, S], bf16, tag=f"t1{g}", bufs=1)
                nc.vector.tensor_tensor(t1[:], rsb[:], pq[:], OP.mult)
                nc.vector.tensor_tensor(pinn[:], t1[:], pinn[:], OP.add)
                n = wrk.tile([R, S], bf16, tag=f"n{g}", bufs=1)
                nc.scalar.activation(n[:], pinn[:], AF.Tanh)
                # z-path (off critical path): u = z*h ; w1 = 1-z
                u = wrk.tile([R, S + 2], bf16, tag=f"u{g}", bufs=1)
                if t == 0:
                    nc.vector.memset(u[:], 0.0)
                nc.gpsimd.tensor_tensor(u[:, 1 : S + 1], zsb[:], hrow0, OP.mult)
                w1 = wrk.tile([R, S], bf16, tag=f"rsb{g}", bufs=1)
                nc.vector.tensor_scalar(w1[:], zsb[:], -1.0, 1.0, op0=OP.mult, op1=OP.add)
                m = wrk.tile([R, S + 2], bf16, tag=f"m{g}", bufs=1)
                if t == 0:
                    nc.vector.memset(m[:], 0.0)
                nc.vector.tensor_tensor(m[:, 1 : S + 1], w1[:], n[:], OP.mult)
                # conv(ctx) = conv(m) + conv(u); ctx = m + u stored off-path
                pc = ps.tile([R, S], fp32, tag="pq", name=f"pc{g}")
                for tap in range(3):
                    nc.tensor.matmul(
                        pc[:], wcv[:, tap, :], u[:, tap : tap + S],
                        start=(tap == 0), stop=False,
                    )
                for tap in range(3):
                    nc.tensor.matmul(
                        pc[:], wcv[:, tap, :], m[:, tap : tap + S],
                        start=False, stop=(tap == 2),
                    )
                nc.scalar.activation(
                    hh[g][0:R, (t + 1) % HS, :], pc[:], AF.Tanh, bias=cvb
                )
                if t >= W:
                    nc.vector.tensor_tensor(
                        ctx[g][:, cslot, 1 : S + 1], m[:, 1 : S + 1], u[:, 1 : S + 1], OP.add
                    )
                if g == 0 and t == W - 1:
                    # chunk 0 starts exactly from h=0 at step 0
                    nc.vector.memset(hh[0][0:24, W % HS, :], 0.0)


        # ---------------- pass 1 (L-part) ----------------
        cp_i = 0
        for tb in range(0, CL, 4):
            for g in range(G):
                lsb4 = wrk.tile([48, 2, 4, S // 2], bf16, tag=f"lsb{g}", bufs=1)
                for k in range(4):
                    tau = tb + k
                    pl = pss[g].tile([48, S], fp32, tag=("pr", "pz", "pinn", "pq")[k], name=f"pl{g}")
                    nc.tensor.matmul(pl[:], wl, ctx[g][:, tau, 1 : S + 1], start=True, stop=True)
                    eng = (nc.vector, nc.scalar)[cp_i % 2]
                    cp_i += 1
                    outv = lsb4[:, :, k, :].rearrange("r p h2 -> r h2 p")
                    inv = pl[:].rearrange("r (h2 p) -> r h2 p", p=2)
                    if eng is nc.scalar:
                        eng.copy(outv, inv)
                    else:
                        eng.tensor_copy(outv, inv)
                for p in range(2):
                    nc.sync.dma_start(
                        out=lp_v[p, JPG * g : JPG * g + JPG, :, :, tb : tb + 4, :],
                        in_=lsb4[:, p, :, :],
                    )

        # ---------------- xbar transpose: lpart[b,o] (S_w, S_h) -> h-major, split by h parity ----------------
        P = min(S // 2, 128)
        J = (S // 2) // P
        for b in range(B):
            for o in range(C):
                for pp in range(2):
                    ltb = wrk.tile([P, J, S], bf16, tag="ltb", bufs=2)
                    nc.sync.dma_start_transpose(ltb[:], lp_d[pp, b, o, :, :])
                    for j in range(J):  # h2 half j: lower half feeds stream 0
                        lpt_v = lpt_d[pp, b, o, j * P : (j + 1) * P, :].rearrange(
                            "(j2 p) w -> p j2 w", p=P
                        )
                        nc.sync.dma_start(out=lpt_v, in_=ltb[:, j : j + 1, :])

        # ---------------- pass 2: A-part + L + sigmoid -> out ----------------
        ptags = ["pr", "pz", "pinn", "pq"]
        for q4 in range(CL // 4):
            for g in range(G):
                l2b = wrk.tile([48, 2, 2, S], bf16, tag=f"l2{g}", bufs=1)
                for p in range(2):
                    nc.sync.dma_start(
                        out=l2b[:, p, :, :],
                        in_=lpt_p4[p, JPG * g : JPG * g + JPG, :, :, q4, :, :],
                    )
                ot4 = wrk.tile([48, 2, 2, S], bf16, tag=f"ot{g}", bufs=1)
                for qq in range(2):
                    for p in range(2):
                        tau = 4 * q4 + 2 * qq + p
                        pa = pss[g].tile([48, S], fp32, tag=ptags[2 * qq + p], name=f"pa{g}")
                        nc.tensor.matmul(pa[:], wa, ctx[g][:, tau, 1 : S + 1], start=True, stop=False)
                        nc.tensor.matmul(pa[:], idt, l2b[:, p, qq, :], start=False, stop=True)
                        nc.scalar.activation(ot4[:, p, qq, :], pa[:], AF.Sigmoid, bias=cbb)
                for p in range(2):
                    nc.sync.dma_start(out=out_d[g, q4, p], in_=ot4[:, p, :, :])
    nc.compile()
    return nc


def _run(x, packed, S, trace=False, nc=None):
    """Shard over 8 cores, run, gather. x: (8B, C, S, S) f32."""
    from concourse.bass_utils import run_bass_kernel_spmd

    if nc is None:
        nc = build_nc(S)
    in_maps = []
    for core in range(NCORES):
        xc = x[core * B : (core + 1) * B]
        in_maps.append({"x2": _pack_x2(xc, S), **packed})
    core_ids = list(range(NCORES))
    res = run_bass_kernel_spmd(nc, in_maps, core_ids, trace=trace)
    outs = []
    Q4 = S // (4 * NCH)
    for i in range(NCORES):
        o2 = np.asarray(res.results[i]["out"], np.float32).reshape(G, Q4, 2, JPG, B, C, 2, S)
        # [g,q4,p,j,b,o,qq,w] -> h = (((g*JPG+j)*Q4+q4)*2+qq)*2+p
        o2 = np.transpose(o2, (4, 5, 0, 3, 1, 6, 2, 7)).reshape(B, C, S, S)
        outs.append(o2)
    return np.concatenate(outs, axis=0), res


def kernel(**inputs):
    x = np.asarray(inputs["x"], np.float32)
    packed = _pack_weights(
        {k: np.asarray(v, np.float32) for k, v in inputs.items() if k != "x"}
    )
    out, _ = _run(x, packed, x.shape[2])
    return out.astype(np.float32)


# revision 3
# speedup vs baseline: 1.0560x; 1.0560x over previous
"""CSRN Trainium2 kernel v2: chunked-parallel scan.

Sharding: data-parallel over batch across 8 cores (4 batches/core).
Each core fuses both directional scans; the 512 scan steps are split
into 8 chunks run in parallel (the recurrence is contractive: W=16
warmup steps from h=0 reproduce the exact state to ~1e-6).

Organization per core: 2 independent instruction streams; each stream
carries 32 chains (4 chunks x [2 scans x 4 batches]) x 3 channels =
96 data rows, positions 0..511 on the free dim. slots/stream =
S/8 + W.  Gate matmuls: M-tiles r/z/inn/hnh (96 rows each); inn has
no h-part and hnh no x-part, so 6 matmuls/slot + 3 conv taps.

Layout rules honored: engine SBUF access patterns start at partition
0/32/64/96 only (PSUM operands exempt); matmul lhsT/rhs share a
32-aligned base partition; one matmul output <= one PSUM bank (N<=512).
"""

import sys

sys.path.insert(0, "/opt/trn_rl_repo")

import numpy as np
import ml_dtypes

BF16 = ml_dtypes.bfloat16

C = 3  # channels
B = 4  # batches per core
NCORES = 8
NCH = 8  # global chunks
G = 2  # streams
JPG = NCH // G  # chunks per stream
BS = 2 * B  # batch-scan chains (scan 0 = rows, 1 = cols)
R = JPG * BS * C  # 96 data rows per stream
W = 8  # warmup steps
XRING = 10  # x ring slots
XCH = 2  # x DMA chunk (slots)
HS = 2  # h ring slots


def _pack_weights(inp):
    """lhsT matrices -> single (128, 1160) bf16 block 'wpk'.

    Row order within a stream: row = 24*j + 3*bs + c, j = chunk-in-stream,
    bs = scan*4 + b (scan 0 = row scan), c = channel.
    Column map: wgA (x+ones rows, 4 gate M-tiles) 0:384; wgB (h rows)
    384:768; conv taps 768:1056; wa 1056:1104; wl 1104:1152;
    conv bias col 1152; combine bias col 1153.
    """
    w_ih = [inp["w_ih_rows"], inp["w_ih_cols"]]
    w_hh = [inp["w_hh_rows"], inp["w_hh_cols"]]
    b_ih = [inp["b_ih_rows"], inp["b_ih_cols"]]
    b_hh = [inp["b_hh_rows"], inp["b_hh_cols"]]
    cv_w = [inp["conv_rows_w"], inp["conv_cols_w"]]
    cv_b = [inp["conv_rows_b"], inp["conv_cols_b"]]
    cb_w = inp["combine_w"]  # (C, 2C)
    cb_b = inp["combine_b"]  # (C,)

    wgA = np.zeros((97, 4 * R), np.float32)  # K = x rows 0..95, ones 96
    wgB = np.zeros((97, 4 * R), np.float32)  # K = h rows 0..95, ones 96
    # gate order in M-tiles: 0=r, 1=z, 2=inn, 3=hnh
    for j in range(JPG):
        for bs in range(BS):
            sc = bs // B
            base = 24 * j + 3 * bs
            for ci in range(C):
                for co in range(C):
                    k, m = base + ci, base + co
                    wgA[k, 0 * R + m] = w_ih[sc][0 + co, ci]
                    wgB[k, 0 * R + m] = w_hh[sc][0 + co, ci]
                    wgA[k, 1 * R + m] = w_ih[sc][3 + co, ci]
                    wgB[k, 1 * R + m] = w_hh[sc][3 + co, ci]
                    wgA[k, 2 * R + m] = w_ih[sc][6 + co, ci]
                    wgB[k, 3 * R + m] = w_hh[sc][6 + co, ci]
            for co in range(C):
                m = base + co
                wgA[96, 0 * R + m] = b_ih[sc][co] + b_hh[sc][co]
                wgA[96, 1 * R + m] = b_ih[sc][3 + co] + b_hh[sc][3 + co]
                wgA[96, 2 * R + m] = b_ih[sc][6 + co]
                wgB[96, 3 * R + m] = b_hh[sc][6 + co]

    wcv = np.zeros((R, 3, R), np.float32)
    for j in range(JPG):
        for bs in range(BS):
            sc = bs // B
            base = 24 * j + 3 * bs
            for t in range(3):
                for ci in range(C):
                    for co in range(C):
                        wcv[base + ci, t, base + co] = cv_w[sc][co, ci, t]

    # combine lhsT: K = 96 ctx rows, M = 48 (j, b, o)
    wa = np.zeros((R, 48), np.float32)
    wl = np.zeros((R, 48), np.float32)
    for j in range(JPG):
        for b in range(B):
            for o in range(C):
                m = 12 * j + 3 * b + o
                for ci in range(C):
                    wa[24 * j + 3 * b + ci, m] = cb_w[o, ci]
                    wl[24 * j + 3 * (4 + b) + ci, m] = cb_w[o, C + ci]

    wpk = np.zeros((128, 1352), np.float32)
    wpk[0:48, 1155:1203] = np.eye(48, dtype=np.float32)
    wpk[64:112, 1203:1251] = np.eye(48, dtype=np.float32)
    wpk[0:96, 1256:1352] = np.eye(96, dtype=np.float32)
    wpk[0:97, 0:384] = wgA
    wpk[0:97, 384:768] = wgB
    wpk[0:R, 768:1056] = wcv.reshape(R, 288)
    wpk[0:R, 1056:1104] = wa
    wpk[0:R, 1104:1152] = wl
    for j in range(JPG):
        for bs in range(BS):
            for co in range(C):
                wpk[24 * j + 3 * bs + co, 1152] = cv_b[bs // B][co]
    for j in range(JPG):
        for b in range(B):
            for o in range(C):
                wpk[12 * j + 3 * b + o, 1153] = cb_b[o]
                for p in range(2):
                    wpk[64 * p + 12 * j + 3 * b + o, 1154] = cb_b[o]
    return {"wpk": wpk.astype(BF16)}


def _pack_x2(x_core, S):
    """x_core: (B, C, S, S) f32 -> (G, 97, slots, S) bf16 matmul rows.

    x2[g, 24j+3bs+c, t, :] = x[b, c, step, :] (row scan, bs<4) or
    x[b, c, :, step] (col scan), step = (S/8)*(JPG*g+j) + t - W; zeros
    for step < 0 (chunk 0 warmup; its h is reset at slot W on device).
    Row 96 = ones.
    """
    CL = S // NCH
    slots = CL + W
    x2 = np.zeros((G, 96, slots, S), np.float32)
    steps = np.arange(slots) - W
    for g in range(G):
        for j in range(JPG):
            cg = JPG * g + j
            st = CL * cg + steps
            valid = st >= 0
            stc = np.clip(st, 0, S - 1)
            for bs in range(BS):
                b = bs % B
                rows = slice(24 * j + 3 * bs, 24 * j + 3 * bs + 3)
                if bs < 4:
                    blk = x_core[b, :, stc, :]  # (slots, C, S)
                else:
                    blk = x_core[b, :, :, stc]  # (slots, C, S)
                blk = np.transpose(blk, (1, 0, 2)) * valid[None, :, None]
                x2[g, rows, :, :] = blk
    return x2.astype(BF16)


def build_nc(S, lt_pmajor=False):
    import concourse.bass as bass
    import concourse.bacc as bacc
    import concourse.mybir as mybir
    from concourse.tile import TileContext
    from contextlib import ExitStack

    fp32 = mybir.dt.float32
    bf16 = mybir.dt.bfloat16
    AF = mybir.ActivationFunctionType
    OP = mybir.AluOpType

    CL = S // NCH  # chunk length (real slots per stream)
    SLOTS = CL + W

    nc = bacc.Bacc()
    x2_d = nc.declare_dram_parameter("x2", [G, 96, SLOTS, S], bf16, isOutput=False)
    wpk_d = nc.declare_dram_parameter("wpk", [128, 1352], bf16, isOutput=False)
    out_d = nc.declare_dram_parameter("out", [G, S // (8 * NCH), 48, 8, S], bf16, isOutput=True)
    lp_d = nc.dram_tensor("lpart", [B, C, S, S], bf16)  # [b, o, w, h]
    lpt_d = nc.dram_tensor("lpartT", [B, C, S, S], bf16)  # [b, o, h, w]

    lp_v = lp_d.rearrange("b o (c wl) h -> c b o wl h", c=NCH)
    lpt_8 = lpt_d.rearrange("b o (c q8 i) w -> c b o q8 i w", c=NCH, i=8)

    with TileContext(nc) as tc, ExitStack() as es:
        cst = es.enter_context(tc.tile_pool(name="cst", bufs=1))
        big = es.enter_context(tc.tile_pool(name="big", bufs=1))
        wrk = es.enter_context(tc.tile_pool(name="wrk", bufs=2))
        pss = [
            es.enter_context(tc.tile_pool(name=f"ps{g}", bufs=1, space="PSUM"))
            for g in range(G)
        ]
        plp = es.enter_context(tc.tile_pool(name="plp", bufs=2, space="PSUM"))

        wpk = cst.tile([128, 1352], bf16)
        nc.sync.dma_start(out=wpk[:], in_=wpk_d[:])
        wg = [wpk[0:97, 0:384], wpk[0:97, 384:768]]  # A (x+ones), B (h+ones)
        wcv = wpk[0:R, 768:1056].rearrange("p (t m) -> p t m", t=3)
        wa = wpk[0:R, 1056:1104]
        wl = wpk[0:R, 1104:1152]
        cvb = wpk[0:R, 1152:1153]
        cbb = wpk[0:48, 1153:1154]
        id96 = wpk[0:96, 1256:1352]
        cbb2 = wpk[0:112, 1154:1155]

        # persistent state per stream
        xts = [big.tile([128, XRING, S], bf16, name=f"xts{g}") for g in range(G)]
        hh = [big.tile([97, HS, S], bf16, name=f"hh{g}") for g in range(G)]
        ctx = [big.tile([R, CL, S + 2], bf16, name=f"ctx{g}") for g in range(G)]
        for g in range(G):
            nc.vector.memset(hh[g][:], 0.0)
            nc.vector.memset(hh[g][96:97, :, :], 1.0)
            nc.vector.memset(xts[g][96:97, :, :], 1.0)
            nc.vector.memset(ctx[g][:, :, 0:1], 0.0)
            nc.vector.memset(ctx[g][:, :, S + 1 : S + 2], 0.0)
            # initial x prefetch: slots [0, 2*XCH)
            for t0 in (0, XCH):
                nc.sync.dma_start(
                    out=xts[g][0:96, t0 : t0 + XCH, :],
                    in_=x2_d[g, :, t0 : t0 + XCH, :],
                )

        # ---------------- scan loop ----------------
        lsb4s = [None, None]
        for t in range(SLOTS):
            for g in range(G):
                ps = pss[g]
                sl = t % XRING
                hsl = t % HS
                cslot = t - W if t >= W else min(t, CL - 1)
                if t % XCH == 0:
                    t0 = t + 2 * XCH
                    if t0 < SLOTS:
                        n_s = min(XCH, SLOTS - t0)
                        nc.sync.dma_start(
                            out=xts[g][0:96, t0 % XRING : t0 % XRING + n_s, :],
                            in_=x2_d[g, :, t0 : t0 + n_s, :],
                        )
                xrow = xts[g][0:97, sl, :]
                hrow = hh[g][0:97, hsl, :]
                hrow0 = hh[g][0:R, hsl, :]

                pr = ps.tile([R, S], fp32, tag="pg", name=f"pr{g}")
                nc.tensor.matmul(pr[:], wg[0][:, 0 * R : 0 * R + R], xrow, start=True, stop=False)
                nc.tensor.matmul(pr[:], wg[1][:, 0 * R : 0 * R + R], hrow, start=False, stop=True)
                rsb = wrk.tile([R, S], bf16, tag=f"rsb{g}", bufs=1)
                nc.scalar.activation(rsb[:], pr[:], AF.Sigmoid)
                pq = ps.tile([R, S], fp32, tag="pq", name=f"pq{g}")  # hnh
                nc.tensor.matmul(pq[:], wg[1][:, 3 * R : 3 * R + R], hrow, start=True, stop=True)
                pinn = ps.tile([R, S], fp32, tag="pinn", name=f"pinn{g}")
                nc.tensor.matmul(pinn[:], wg[0][:, 2 * R : 2 * R + R], xrow, start=True, stop=True)
                # n-path: pinn += r * hnh
                t1 = wrk.tile([R, S], bf16, tag=f"t1{g}", bufs=1)
                nc.vector.tensor_tensor(t1[:], rsb[:], pq[:], OP.mult)
                nc.vector.tensor_tensor(pinn[:], t1[:], pinn[:], OP.add)
                n = wrk.tile([R, S], bf16, tag=f"t1{g}", bufs=1)
                nc.scalar.activation(n[:], pinn[:], AF.Tanh)
                # z gates reuse the pg bank after sigma_r consumed it
                pz = ps.tile([R, S], fp32, tag="pg", name=f"pz{g}")
                nc.tensor.matmul(pz[:], wg[0][:, 1 * R : 1 * R + R], xrow, start=True, stop=False)
                nc.tensor.matmul(pz[:], wg[1][:, 1 * R : 1 * R + R], hrow, start=False, stop=True)
                zsb = wrk.tile([R, S], bf16, tag=f"zsb{g}", bufs=1)
                nc.scalar.activation(zsb[:], pz[:], AF.Sigmoid)
                # z-path: u = z*h ; w1 = 1-z ; ctx = w1*n + u
                u = wrk.tile([R, S], bf16, tag=f"u{g}", bufs=1)
                nc.gpsimd.tensor_tensor(u[:], zsb[:], hrow0, OP.mult)
                w1 = wrk.tile([R, S], bf16, tag=f"rsb{g}", bufs=1)
                nc.vector.tensor_scalar(w1[:], zsb[:], -1.0, 1.0, op0=OP.mult, op1=OP.add)
                m = wrk.tile([R, S], bf16, tag=f"m{g}", bufs=1)
                nc.vector.tensor_tensor(m[:], w1[:], n[:], OP.mult)
                nc.vector.tensor_tensor(
                    ctx[g][:, cslot, 1 : S + 1], m[:], u[:], OP.add
                )
                # conv + tanh -> next h
                pc = ps.tile([R, S], fp32, tag="pq", name=f"pc{g}")
                for tap in range(3):
                    nc.tensor.matmul(
                        pc[:], wcv[:, tap, :], ctx[g][:, cslot, tap : tap + S],
                        start=(tap == 0), stop=(tap == 2),
                    )
                nc.scalar.activation(
                    hh[g][0:R, (t + 1) % HS, :], pc[:], AF.Tanh, bias=cvb
                )
                if t >= W:
                    tau = t - W
                    k = tau % 4
                    if k == 0:
                        lsb4s[g] = wrk.tile(
                            [48, 4, S], bf16, tag=f"lsb{g}", bufs=1,
                            name=f"lsb4{g}",
                        )
                    lsb4 = lsb4s[g]
                    pl = plp.tile([48, S], fp32, tag="pl", name="pl")
                    nc.tensor.matmul(pl[:], wl, ctx[g][:, tau, 1 : S + 1], start=True, stop=True)
                    if (tau * G + g) % 2 == 0:
                        nc.vector.tensor_copy(lsb4[:, k, :], pl[:])
                    else:
                        nc.scalar.copy(lsb4[:, k, :], pl[:])
                    if k == 3:
                        nc.sync.dma_start(
                            out=lp_v[JPG * g : JPG * g + JPG, :, :, tau - 3 : tau + 1, :],
                            in_=lsb4[:],
                        )
                if g == 0 and t == W - 1:
                    # chunk 0 starts exactly from h=0 at step 0
                    nc.vector.memset(hh[0][0:24, W % HS, :], 0.0)


        # ---------------- xbar transpose: lpart[b,o] (S_w, S_h) -> h-major, split by h parity ----------------
        P = min(S, 128)
        J = S // P
        for b in range(B):
            for o in range(C):
                ltb = wrk.tile([P, J, S], bf16, tag="ltb", bufs=2)
                nc.sync.dma_start_transpose(ltb[:], lp_d[b, o, :, :])
                if lt_pmajor:  # h = p*J + j
                    lpt_v = lpt_d[b, o].rearrange("(p j) w -> p j w", j=J)
                else:  # h = j*P + p
                    lpt_v = lpt_d[b, o].rearrange("(j p) w -> p j w", p=P)
                nc.sync.dma_start(out=lpt_v, in_=ltb[:])

        # ---------------- pass 2: A-part + L + sigmoid -> out ----------------
        ptags = ["pg", "pinn", "pq"]
        for q8 in range(CL // 8):
            l2b = wrk.tile([112, 8, S], bf16, tag="l2", bufs=2)
            for g in range(G):
                nc.sync.dma_start(
                    out=l2b[64 * g : 64 * g + 48, :, :],
                    in_=lpt_8[JPG * g : JPG * g + JPG, :, :, q8, :, :],
                )
            ot8 = wrk.tile([112, 8, S], bf16, tag="ot", bufs=1)
            for i in range(8):
                tau = 8 * q8 + i
                pa = pss[0].tile([112, S], fp32, tag=ptags[i % 3], name="pa")
                nc.tensor.matmul(pa[0:48, :], wa, ctx[0][:, tau, 1 : S + 1], start=True, stop=True)
                nc.tensor.matmul(pa[64:112, :], wa, ctx[1][:, tau, 1 : S + 1], start=True, stop=True)
                nc.vector.tensor_tensor(pa[:], pa[:], l2b[:, i, :], OP.add)
                nc.scalar.activation(ot8[:, i, :], pa[:], AF.Sigmoid, bias=cbb2)
            for g in range(G):
                nc.sync.dma_start(
                    out=out_d[g, q8], in_=ot8[64 * g : 64 * g + 48, :, :]
                )
    nc.compile()
    return nc


def _run(x, packed, S, trace=False, nc=None):
    """Shard over 8 cores, run, gather. x: (8B, C, S, S) f32."""
    from concourse.bass_utils import run_bass_kernel_spmd

    if nc is None:
        nc = build_nc(S)
    in_maps = []
    for core in range(NCORES):
        xc = x[core * B : (core + 1) * B]
        in_maps.append({"x2": _pack_x2(xc, S), **packed})
    core_ids = list(range(NCORES))
    res = run_bass_kernel_spmd(nc, in_maps, core_ids, trace=trace)
    outs = []
    Q8 = S // (8 * NCH)
    for i in range(NCORES):
        o2 = np.asarray(res.results[i]["out"], np.float32).reshape(G, Q8, JPG, B, C, 8, S)
        # [g,q8,j,b,o,i,w] -> h = 64*(4g+j) + 8*q8 + i
        o2 = np.transpose(o2, (3, 4, 0, 2, 1, 5, 6)).reshape(B, C, S, S)
        outs.append(o2)
    return np.concatenate(outs, axis=0), res


def kernel(**inputs):
    x = np.asarray(inputs["x"], np.float32)
    packed = _pack_weights(
        {k: np.asarray(v, np.float32) for k, v in inputs.items() if k != "x"}
    )
    out, _ = _run(x, packed, x.shape[2])
    return out.astype(np.float32)
